# revision 23
# baseline (speedup 1.0000x reference)
"""EntropyGuidedAttention Trainium2 Bass kernel.

Strategy (data-parallel over batch, 2 batches per core on 8 cores):

All compute stays in the DRAM-native [feature, token] orientation:
  visual_feat[b] (= vf.T, [D, N]) is both the rhs of the q-projection and
  the input of the feature-entropy pass; attention is computed transposed
  (A.T = [Q, N]) so the softmax-over-Q reductions become ones-vector
  matmuls on the PE, and the AV product directly yields the [D, N] output
  layout. No per-tile transposes anywhere in the streaming loop.

Entropy uses ent = log(Z) - T/Z with Z = sum(e^x), T = sum(x e^x)
(no elementwise log). The token softmaxes skip max-subtraction: the
entropy-modulated logits are O(1e-5) and feature logits are N(0,1), so
exp() is safe in fp32.

Matmuls run in float32r (1 cycle/row at free-dim >= 256, fp32-equivalent
precision as measured on HW). qT/kT are stored fp8-e4m3 (they feed only
the modulated-logit path, where the ve*te factor ~1e-6 crushes rounding
error); fp8 halves their SBUF so both batches' qT can be live at once,
letting batch b+1's projections overlap batch b's attention phase
(instruction emission is interleaved per group to make that possible on
the in-order engines). The AV product and v stay float32r.

B=16, D=768, HxW=4096 tokens, Q=128.
"""

from contextlib import ExitStack

import numpy as np

import concourse.bacc as bacc
import concourse.mybir as mybir
import concourse.tile as tile
from concourse.bass import ts
from concourse.bass_utils import run_bass_kernel_spmd
from concourse.masks import make_identity

F32 = mybir.dt.float32
F32R = mybir.dt.float32r
BF16 = mybir.dt.bfloat16
FP8 = mybir.dt.float8e4
AF = mybir.ActivationFunctionType

N_CORES = 8
B, D, HH, WW, Q = 16, 768, 64, 64, 128
N = HH * WW                    # 4096 tokens per batch
BPC = B // N_CORES             # 2 batches per core
DC = D // 128                  # 6 feature chunks
G = 512                        # token group width
NG = N // G                    # 8 groups per batch
SQRT_D = float(np.sqrt(np.float32(D)))


def build_bass():
    nc = bacc.Bacc(None, target_bir_lowering=False)

    visual = nc.dram_tensor("visual", [BPC, D, N], F32R, kind="ExternalInput")
    text = nc.dram_tensor("text", [BPC, Q, D], F32R, kind="ExternalInput")
    wq = nc.dram_tensor("wq", [D, D], F32, kind="ExternalInput")
    wk = nc.dram_tensor("wk", [D, D], F32, kind="ExternalInput")
    wv = nc.dram_tensor("wv", [D, D], F32, kind="ExternalInput")
    bq = nc.dram_tensor("bq", [D], F32, kind="ExternalInput")
    bk = nc.dram_tensor("bk", [D], F32, kind="ExternalInput")
    bv = nc.dram_tensor("bv", [D], F32R, kind="ExternalInput")
    out = nc.dram_tensor("out", [BPC, D, N], F32, kind="ExternalOutput")
    ve_dram = nc.dram_tensor("ve_scratch", [BPC, NG, G], F32)
    c0_dram = nc.dram_tensor("c0_scratch", [BPC, 1, 1], F32)
    st_dram = nc.dram_tensor("st_scratch", [BPC, 1, 128], F32)

    with tile.TileContext(nc) as tc, ExitStack() as ctx:
        K(ctx, tc, visual, text, wq, wk, wv, bq, bk, bv, out,
          ve_dram, c0_dram, st_dram).emit()
    return nc


class K:
    def __init__(self, ctx, tc, visual, text, wq, wk, wv, bq, bk, bv, out,
                 ve_dram, c0_dram, st_dram):
        self.ctx, self.tc, self.nc = ctx, tc, tc.nc
        self.visual, self.text = visual, text
        self.wq, self.wk, self.wv = wq, wk, wv
        self.bq, self.bk, self.bv = bq, bk, bv
        self.out = out
        self.ve_dram, self.c0_dram, self.st_dram = ve_dram, c0_dram, st_dram
        self.st = [dict() for _ in range(BPC)]   # per-batch tile state

    def emit(self):
        self.preamble()
        self.prebatch(0)
        for g in range(NG):
            self.phase1_group(0, g)
        self.finalize(0)
        self.prebatch(1)
        for g in range(NG):
            self.phase2_group(0, g)
            self.phase1_group(1, g)
        self.finalize(1)
        for g in range(NG):
            self.phase2_group(1, g)

    # ---------------- one-time preamble ----------------
    def preamble(self):
        nc, tc, ctx = self.nc, self.tc, self.ctx
        persist = ctx.enter_context(tc.tile_pool(name="persist", bufs=1))
        self.persist = persist

        ident = persist.tile([128, 128], F32, tag="ident")
        make_identity(nc, ident)
        self.ident = ident
        ones_col_f = persist.tile([128, 1], F32, tag="ones_col_f")
        nc.vector.memset(ones_col_f, 1.0)
        ones_col = persist.tile([128, 1], F32R, tag="ones_col")
        nc.scalar.copy(out=ones_col, in_=ones_col_f)
        self.ones_col = ones_col
        ones_row_f = persist.tile([1, 128], F32, tag="ones_row_f")
        nc.vector.memset(ones_row_f, 1.0)
        ones_row = persist.tile([1, 128], F32R, tag="ones_row")
        nc.scalar.copy(out=ones_row, in_=ones_row_f)
        self.ones_row = ones_row

        self.bq_col = persist.tile([128, DC], F32, tag="bq_col")
        nc.sync.dma_start(out=self.bq_col,
                          in_=self.bq.ap().rearrange("(c p) -> p c", p=128))
        self.bk_col = persist.tile([128, DC], F32, tag="bk_col")
        nc.sync.dma_start(out=self.bk_col,
                          in_=self.bk.ap().rearrange("(c p) -> p c", p=128))
        self.bv_row = persist.tile([1, D], F32R, tag="bv_row")
        nc.sync.dma_start(out=self.bv_row,
                          in_=self.bv.ap().rearrange("(a k) -> a k", a=1))

        # transpose the three weight matrices via PE
        self.wqT = persist.tile([128, DC, D], F32R, tag="wqT")
        self.wkT = persist.tile([128, DC, D], F32R, tag="wkT")
        self.wvT = persist.tile([128, DC, D], F32R, tag="wvT")
        with tc.tile_pool(name="pre_sb", bufs=2) as pre_sb, \
             tc.tile_pool(name="pre_ps", bufs=3, space="PSUM") as pre_ps:
            for w_dram, wT in ((self.wq, self.wqT), (self.wk, self.wkT),
                               (self.wv, self.wvT)):
                w_nat = pre_sb.tile([128, DC, D], F32, tag="w_nat")
                nc.sync.dma_start(
                    out=w_nat,
                    in_=w_dram.ap().rearrange("(c p) k -> p c k", p=128),
                )
                for jc in range(DC):
                    for kc in range(DC):
                        pt = pre_ps.tile([128, 128], F32, tag="pt")
                        nc.tensor.transpose(pt, w_nat[:, jc, ts(kc, 128)], ident)
                        nc.scalar.copy(out=wT[:, kc, ts(jc, 128)], in_=pt)

        # streaming pools
        self.vf_pool = ctx.enter_context(tc.tile_pool(name="vf", bufs=2))
        self.es_pool = ctx.enter_context(tc.tile_pool(name="escr", bufs=3))
        self.at_pool = ctx.enter_context(tc.tile_pool(name="attn", bufs=2))
        self.oc_pool = ctx.enter_context(tc.tile_pool(name="outc", bufs=2))
        self.sm_pool = ctx.enter_context(tc.tile_pool(name="small", bufs=1))
        self.pb_pool = ctx.enter_context(tc.tile_pool(name="perbatch", bufs=1))
        self.pb2_pool = ctx.enter_context(tc.tile_pool(name="perbatch2", bufs=2))
        self.mm_ps = ctx.enter_context(tc.tile_pool(name="mm_ps", bufs=3, space="PSUM"))
        self.zt_ps = ctx.enter_context(tc.tile_pool(name="zt_ps", bufs=3, space="PSUM"))
        self.lg_ps = ctx.enter_context(tc.tile_pool(name="lg_ps", bufs=2, space="PSUM"))

    # ---------------- per-batch text preamble: textT, te, kT, v ----------------
    def prebatch(self, b):
        nc = self.nc
        st = self.st[b]
        text_nat = self.pb_pool.tile([Q, D], F32R, tag="text_nat", name=f"text_nat{b}")
        nc.sync.dma_start(out=text_nat, in_=self.text.ap()[b])
        text_f = text_nat.bitcast(F32)

        textT = self.pb_pool.tile([128, DC, Q], F32R, tag="textT", name=f"textT{b}")
        for dc in range(DC):
            pt = self.mm_ps.tile([128, G], F32, tag="mm")
            nc.tensor.transpose(pt[:, :Q], text_f[:, ts(dc, 128)], self.ident)
            nc.scalar.copy(out=textT[:, dc, :], in_=pt[:, :Q])

        # text entropy -> evt (unnormalized te), S_t
        sm = self.sm_pool
        maxm = sm.tile([Q, 1], F32, tag="maxm")
        nc.vector.reduce_max(out=maxm, in_=text_f, axis=mybir.AxisListType.X)
        negm = sm.tile([Q, 1], F32, tag="negm")
        nc.vector.tensor_scalar_mul(out=negm, in0=maxm, scalar1=-1.0)
        et = self.es_pool.tile([Q, D], F32, tag="ex", name=f"et{b}")
        zt = sm.tile([Q, 1], F32, tag="zt")
        nc.scalar.activation(out=et, in_=text_f, func=AF.Exp, bias=negm, accum_out=zt)
        tt = sm.tile([Q, 1], F32, tag="tt")
        nc.vector.tensor_mul(out=et, in0=et, in1=text_f)
        nc.vector.reduce_sum(out=tt, in_=et, axis=mybir.AxisListType.X)
        rzt = sm.tile([Q, 1], F32, tag="rzt")
        nc.vector.reciprocal(out=rzt, in_=zt)
        t2 = sm.tile([Q, 1], F32, tag="t2")
        nc.vector.tensor_mul(out=t2, in0=tt, in1=rzt)
        lnz = sm.tile([Q, 1], F32, tag="lnz")
        nc.scalar.activation(out=lnz, in_=zt, func=AF.Ln)
        ent_t = sm.tile([Q, 1], F32, tag="ent_t")
        nc.vector.tensor_sub(out=ent_t, in0=lnz, in1=t2)
        nc.vector.tensor_add(out=ent_t, in0=ent_t, in1=maxm)
        evt = sm.tile([Q, 1], F32, tag="evt", name=f"evt{b}")
        nc.scalar.activation(out=evt, in_=ent_t, func=AF.Exp)
        st["evt"] = evt
        # S_t via DRAM round-trip (column -> row)
        nc.sync.dma_start(
            out=self.st_dram.ap()[b].rearrange("one p -> p one"), in_=evt)
        st_row = sm.tile([1, Q], F32, tag="st_row", name=f"strow{b}")
        nc.sync.dma_start(out=st_row, in_=self.st_dram.ap()[b])
        st_sb = sm.tile([1, 1], F32, tag="st_sb", name=f"stsb{b}")
        nc.vector.reduce_sum(out=st_sb, in_=st_row, axis=mybir.AxisListType.X)
        st["st_sb"] = st_sb

        # kT projection (fp8, j on partitions)
        kTb = self.pb2_pool.tile([128, DC, Q], FP8, tag="kTb", name=f"kTb{b}")
        for jc in range(DC):
            kp = self.mm_ps.tile([128, G], F32, tag="mm")
            for dc in range(DC):
                nc.tensor.matmul(
                    kp[:, :Q], self.wkT[:, dc, ts(jc, 128)], textT[:, dc, :],
                    start=(dc == 0), stop=(dc == DC - 1),
                )
            nc.scalar.activation(
                out=kTb[:, jc, :], in_=kp[:, :Q], func=AF.Identity,
                bias=self.bk_col[:, jc : jc + 1],
            )
        st["kTb"] = kTb

        # v projection (float32r, q on partitions)
        v_sb = self.pb2_pool.tile([Q, D], F32R, tag="v_sb", name=f"v{b}")
        for jg, jw in ((0, G), (1, D - G)):
            vp = self.mm_ps.tile([128, G], F32, tag="mm")
            for dc in range(DC):
                nc.tensor.matmul(
                    vp[:, :jw], textT[:, dc, :],
                    self.wvT[:, dc, jg * G : jg * G + jw],
                    start=(dc == 0), stop=False,
                )
            nc.tensor.matmul(
                vp[:, :jw], self.ones_row, self.bv_row[:, jg * G : jg * G + jw],
                start=False, stop=True,
            )
            nc.scalar.copy(out=v_sb[:, jg * G : jg * G + jw], in_=vp[:, :jw])
        st["v_sb"] = v_sb

        st["qT"] = self.pb2_pool.tile([128, DC, N], FP8, tag="qT", name=f"qT{b}")
        st["zc"] = self.pb_pool.tile([NG, G], F32, tag="zc", name=f"zc{b}")
        st["tcol"] = self.pb_pool.tile([NG, G], F32, tag="tcol", name=f"tcol{b}")

    # ---------------- phase 1 (per group): entropy partials + qT ----------------
    def phase1_group(self, b, g):
        nc = self.nc
        st = self.st[b]
        gs = slice(g * G, (g + 1) * G)
        vf = self.vf_pool.tile([128, DC, G], F32R, tag="vf")
        nc.sync.dma_start(
            out=vf,
            in_=self.visual.ap()[b].rearrange("(c p) n -> p c n", p=128)[:, :, gs],
        )
        vf_f = vf.bitcast(F32)

        zp = self.zt_ps.tile([1, G], F32, tag="zt")
        tp = self.zt_ps.tile([1, G], F32, tag="zt")
        for dc in range(DC):
            ex = self.es_pool.tile([128, G], F32R, tag="ex")
            nc.scalar.activation(out=ex, in_=vf_f[:, dc, :], func=AF.Exp)
            xe = self.es_pool.tile([128, G], F32R, tag="xe")
            nc.vector.tensor_mul(out=xe, in0=ex.bitcast(F32), in1=vf_f[:, dc, :])
            nc.tensor.matmul(zp, self.ones_col, ex,
                             start=(dc == 0), stop=(dc == DC - 1))
            nc.tensor.matmul(tp, self.ones_col, xe,
                             start=(dc == 0), stop=(dc == DC - 1))

        zrow = self.sm_pool.tile([1, G], F32, tag="zrow")
        nc.scalar.copy(out=zrow, in_=zp)
        nc.sync.dma_start(out=st["zc"][g : g + 1, :], in_=zrow)
        trow = self.sm_pool.tile([1, G], F32, tag="trow")
        nc.scalar.copy(out=trow, in_=tp)
        nc.sync.dma_start(out=st["tcol"][g : g + 1, :], in_=trow)

        for jc in range(DC):
            qp = self.mm_ps.tile([128, G], F32, tag="mm")
            for dc in range(DC):
                nc.tensor.matmul(
                    qp, self.wqT[:, dc, ts(jc, 128)], vf[:, dc, :],
                    start=(dc == 0), stop=(dc == DC - 1),
                )
            nc.vector.tensor_scalar_add(
                out=st["qT"][:, jc, gs], in0=qp,
                scalar1=self.bq_col[:, jc : jc + 1],
            )

    # ---------------- per-batch entropy finalize ----------------
    def finalize(self, b):
        nc = self.nc
        st = self.st[b]
        zc, tcol = st["zc"], st["tcol"]
        rz = self.pb_pool.tile([NG, G], F32, tag="rz", name=f"rz{b}")
        nc.vector.reciprocal(out=rz, in_=zc)
        nc.vector.tensor_mul(out=rz, in0=tcol, in1=rz)
        nc.scalar.activation(out=zc, in_=zc, func=AF.Ln)
        nc.vector.tensor_sub(out=zc, in0=zc, in1=rz)
        exp_ent = self.pb_pool.tile([NG, G], F32R, tag="exp_ent", name=f"ee{b}")
        nc.scalar.activation(out=exp_ent, in_=zc, func=AF.Exp)
        nc.sync.dma_start(out=self.ve_dram.ap()[b], in_=exp_ent.bitcast(F32))

        sve_p = self.zt_ps.tile([1, G], F32, tag="zt")
        nc.tensor.matmul(sve_p, self.ones_col[:NG], exp_ent, start=True, stop=True)
        sve_sb = self.sm_pool.tile([1, 1], F32, tag="sve_sb", name=f"sve{b}")
        nc.vector.reduce_sum(out=sve_sb, in_=sve_p, axis=mybir.AxisListType.X)

        c0 = self.sm_pool.tile([1, 1], F32, tag="c0", name=f"c0{b}")
        nc.vector.tensor_mul(out=c0, in0=st["st_sb"], in1=sve_sb)
        nc.vector.reciprocal(out=c0, in_=c0)
        nc.vector.tensor_scalar_mul(out=c0, in0=c0, scalar1=1.0 / SQRT_D)
        nc.sync.dma_start(out=self.c0_dram.ap()[b], in_=c0)
        c0b = self.sm_pool.tile([128, 1], F32, tag="c0b", name=f"c0b{b}")
        nc.sync.dma_start(out=c0b, in_=self.c0_dram.ap()[b].broadcast_to((128, 1)))
        te_eff = self.pb2_pool.tile([Q, 1], F32, tag="te_eff", name=f"te{b}")
        nc.vector.tensor_mul(out=te_eff, in0=st["evt"], in1=c0b)
        st["te_eff"] = te_eff

    # ---------------- phase 2 (per group): attention ----------------
    def phase2_group(self, b, g):
        nc = self.nc
        st = self.st[b]
        gs = slice(g * G, (g + 1) * G)
        veb = self.at_pool.tile([128, G], F32, tag="veb", bufs=4)
        nc.sync.dma_start(
            out=veb, in_=self.ve_dram.ap()[b][g : g + 1, :].broadcast_to((128, G))
        )

        lp = self.lg_ps.tile([Q, G], F32, tag="lg")
        for jc in range(DC):
            nc.tensor.matmul(
                lp, st["kTb"][:, jc, :], st["qT"][:, jc, gs],
                start=(jc == 0), stop=(jc == DC - 1),
            )
        smod = self.at_pool.tile([Q, G], F32, tag="smod")
        nc.vector.tensor_mul(out=smod, in0=lp, in1=veb)
        ea = self.at_pool.tile([Q, G], F32R, tag="ea")
        nc.scalar.activation(out=ea, in_=smod, func=AF.Exp, scale=st["te_eff"])

        zap = self.zt_ps.tile([1, G], F32, tag="zt")
        nc.tensor.matmul(zap, self.ones_col, ea, start=True, stop=True)
        zarow = self.sm_pool.tile([1, G], F32R, tag="zarow")
        nc.scalar.copy(out=zarow, in_=zap)
        zb = self.lg_ps.tile([128, G], F32, tag="lg")
        nc.tensor.matmul(zb, self.ones_row, zarow, start=True, stop=True)
        rzb = self.at_pool.tile([128, G], F32, tag="rzb")
        nc.vector.reciprocal(out=rzb, in_=zb)
        # fold 1/Za into the attention weights once (vs 6 per-j evac muls)
        ean = self.at_pool.tile([Q, G], F32R, tag="smod")
        nc.vector.tensor_mul(out=ean, in0=ea.bitcast(F32), in1=rzb)

        for jh in range(2):
            oc = self.oc_pool.tile([128, DC // 2, G], F32, tag="oc")
            for jx in range(DC // 2):
                jc = jh * (DC // 2) + jx
                ep = self.mm_ps.tile([128, G], F32, tag="mm")
                nc.tensor.matmul(ep, st["v_sb"][:, ts(jc, 128)], ean,
                                 start=True, stop=True)
                nc.scalar.copy(out=oc[:, jx, :], in_=ep)
            nc.sync.dma_start(
                out=self.out.ap()[b].rearrange("(c p) n -> p c n", p=128)[
                    :, jh * (DC // 2) : (jh + 1) * (DC // 2), gs
                ],
                in_=oc,
            )


_compiled = {}


def kernel(**inputs):
    visual_feat = np.ascontiguousarray(inputs["visual_feat"], dtype=np.float32)
    text_feat = np.ascontiguousarray(inputs["text_feat"], dtype=np.float32)
    Wq = np.ascontiguousarray(inputs["Wq"], dtype=np.float32)
    Wk = np.ascontiguousarray(inputs["Wk"], dtype=np.float32)
    Wv = np.ascontiguousarray(inputs["Wv"], dtype=np.float32)
    bq = np.ascontiguousarray(inputs["bq"], dtype=np.float32)
    bk = np.ascontiguousarray(inputs["bk"], dtype=np.float32)
    bv = np.ascontiguousarray(inputs["bv"], dtype=np.float32)

    vis = visual_feat.reshape(B, D, N)
    in_maps = []
    for c in range(N_CORES):
        bs = slice(c * BPC, (c + 1) * BPC)
        in_maps.append(
            {
                "visual": np.ascontiguousarray(vis[bs]),
                "text": np.ascontiguousarray(text_feat[bs]),
                "wq": Wq, "wk": Wk, "wv": Wv,
                "bq": bq, "bk": bk, "bv": bv,
            }
        )

    if "nc" not in _compiled:
        nc = build_bass()
        nc.compile()
        _compiled["nc"] = nc
    res = run_bass_kernel_spmd(_compiled["nc"], in_maps, core_ids=list(range(N_CORES)))
    _compiled["last_result"] = res

    out = np.concatenate([r["out"] for r in res.results], axis=0)
    return out.reshape(B, D, HH, WW)


if __name__ == "__main__":
    nc = build_bass()
    nc.compile()
    print("build ok")


# revision 24
# speedup vs baseline: 1.0157x; 1.0157x over previous
"""EntropyGuidedAttention Trainium2 Bass kernel.

Strategy (data-parallel over batch, 2 batches per core on 8 cores):

All compute stays in the DRAM-native [feature, token] orientation:
  visual_feat[b] (= vf.T, [D, N]) is both the rhs of the q-projection and
  the input of the feature-entropy pass; attention is computed transposed
  (A.T = [Q, N]) so the softmax-over-Q reductions become ones-vector
  matmuls on the PE, and the AV product directly yields the [D, N] output
  layout. No per-tile transposes anywhere in the streaming loop.

Entropy uses ent = log(Z) - T/Z with Z = sum(e^x), T = sum(x e^x)
(no elementwise log). The token softmaxes skip max-subtraction: the
entropy-modulated logits are O(1e-5) and feature logits are N(0,1), so
exp() is safe in fp32.

Matmuls run in float32r (1 cycle/row at free-dim >= 256, fp32-equivalent
precision as measured on HW). qT/kT are stored fp8-e4m3 (they feed only
the modulated-logit path, where the ve*te factor ~1e-6 crushes rounding
error); fp8 halves their SBUF so both batches' qT can be live at once,
letting batch b+1's projections overlap batch b's attention phase
(instruction emission is interleaved per group to make that possible on
the in-order engines). The AV product and v stay float32r.

B=16, D=768, HxW=4096 tokens, Q=128.
"""

from contextlib import ExitStack

import numpy as np

import concourse.bacc as bacc
import concourse.mybir as mybir
import concourse.tile as tile
from concourse.bass import ts
from concourse.bass_utils import run_bass_kernel_spmd
from concourse.masks import make_identity

F32 = mybir.dt.float32
F32R = mybir.dt.float32r
BF16 = mybir.dt.bfloat16
FP8 = mybir.dt.float8e4
AF = mybir.ActivationFunctionType

N_CORES = 8
B, D, HH, WW, Q = 16, 768, 64, 64, 128
N = HH * WW                    # 4096 tokens per batch
BPC = B // N_CORES             # 2 batches per core
DC = D // 128                  # 6 feature chunks
G = 512                        # token group width
NG = N // G                    # 8 groups per batch
SQRT_D = float(np.sqrt(np.float32(D)))


def build_bass():
    nc = bacc.Bacc(None, target_bir_lowering=False)

    visual = nc.dram_tensor("visual", [BPC, D, N], F32R, kind="ExternalInput")
    text = nc.dram_tensor("text", [BPC, Q, D], F32R, kind="ExternalInput")
    wq = nc.dram_tensor("wq", [D, D], F32, kind="ExternalInput")
    wk = nc.dram_tensor("wk", [D, D], F32, kind="ExternalInput")
    wv = nc.dram_tensor("wv", [D, D], F32, kind="ExternalInput")
    bq = nc.dram_tensor("bq", [D], F32, kind="ExternalInput")
    bk = nc.dram_tensor("bk", [D], F32, kind="ExternalInput")
    bv = nc.dram_tensor("bv", [D], F32R, kind="ExternalInput")
    out = nc.dram_tensor("out", [BPC, D, N], F32, kind="ExternalOutput")
    ve_dram = nc.dram_tensor("ve_scratch", [BPC, NG, G], F32)
    c0_dram = nc.dram_tensor("c0_scratch", [BPC, 1, 1], F32)
    st_dram = nc.dram_tensor("st_scratch", [BPC, 1, 128], F32)

    with tile.TileContext(nc) as tc, ExitStack() as ctx:
        K(ctx, tc, visual, text, wq, wk, wv, bq, bk, bv, out,
          ve_dram, c0_dram, st_dram).emit()
    return nc


class K:
    def __init__(self, ctx, tc, visual, text, wq, wk, wv, bq, bk, bv, out,
                 ve_dram, c0_dram, st_dram):
        self.ctx, self.tc, self.nc = ctx, tc, tc.nc
        self.visual, self.text = visual, text
        self.wq, self.wk, self.wv = wq, wk, wv
        self.bq, self.bk, self.bv = bq, bk, bv
        self.out = out
        self.ve_dram, self.c0_dram, self.st_dram = ve_dram, c0_dram, st_dram
        self.st = [dict() for _ in range(BPC)]   # per-batch tile state

    def emit(self):
        self.preamble()
        self.prebatch(0)
        for g in range(NG):
            self.phase1_group(0, g)
        self.finalize(0)
        self.prebatch(1)
        for g in range(NG):
            self.phase2_group(0, g)
            self.phase1_group(1, g)
        self.finalize(1)
        for g in range(NG):
            self.phase2_group(1, g)

    # ---------------- one-time preamble ----------------
    def preamble(self):
        nc, tc, ctx = self.nc, self.tc, self.ctx
        persist = ctx.enter_context(tc.tile_pool(name="persist", bufs=1))
        self.persist = persist

        ident = persist.tile([128, 128], F32, tag="ident")
        make_identity(nc, ident)
        self.ident = ident
        ones_col_f = persist.tile([128, 1], F32, tag="ones_col_f")
        nc.vector.memset(ones_col_f, 1.0)
        ones_col = persist.tile([128, 1], F32R, tag="ones_col")
        nc.scalar.copy(out=ones_col, in_=ones_col_f)
        self.ones_col = ones_col
        ones_row_f = persist.tile([1, 128], F32, tag="ones_row_f")
        nc.vector.memset(ones_row_f, 1.0)
        ones_row = persist.tile([1, 128], F32R, tag="ones_row")
        nc.scalar.copy(out=ones_row, in_=ones_row_f)
        self.ones_row = ones_row

        self.bq_col = persist.tile([128, DC], F32, tag="bq_col")
        nc.sync.dma_start(out=self.bq_col,
                          in_=self.bq.ap().rearrange("(c p) -> p c", p=128))
        self.bk_col = persist.tile([128, DC], F32, tag="bk_col")
        nc.sync.dma_start(out=self.bk_col,
                          in_=self.bk.ap().rearrange("(c p) -> p c", p=128))
        self.bv_row = persist.tile([1, D], F32R, tag="bv_row")
        nc.sync.dma_start(out=self.bv_row,
                          in_=self.bv.ap().rearrange("(a k) -> a k", a=1))

        # transpose the three weight matrices via PE
        self.wqT = persist.tile([128, DC, D], F32R, tag="wqT")
        self.wkT = persist.tile([128, DC, D], F32R, tag="wkT")
        self.wvT = persist.tile([128, DC, D], F32R, tag="wvT")
        with tc.tile_pool(name="pre_sb", bufs=2) as pre_sb, \
             tc.tile_pool(name="pre_ps", bufs=3, space="PSUM") as pre_ps:
            for w_dram, wT in ((self.wq, self.wqT), (self.wk, self.wkT),
                               (self.wv, self.wvT)):
                w_nat = pre_sb.tile([128, DC, D], F32, tag="w_nat")
                nc.sync.dma_start(
                    out=w_nat,
                    in_=w_dram.ap().rearrange("(c p) k -> p c k", p=128),
                )
                for jc in range(DC):
                    for kc in range(DC):
                        pt = pre_ps.tile([128, 128], F32, tag="pt")
                        nc.tensor.transpose(pt, w_nat[:, jc, ts(kc, 128)], ident)
                        nc.scalar.copy(out=wT[:, kc, ts(jc, 128)], in_=pt)

        # streaming pools
        self.vf_pool = ctx.enter_context(tc.tile_pool(name="vf", bufs=2))
        self.es_pool = ctx.enter_context(tc.tile_pool(name="escr", bufs=3))
        self.at_pool = ctx.enter_context(tc.tile_pool(name="attn", bufs=2))
        self.oc_pool = ctx.enter_context(tc.tile_pool(name="outc", bufs=2))
        self.sm_pool = ctx.enter_context(tc.tile_pool(name="small", bufs=1))
        self.pb_pool = ctx.enter_context(tc.tile_pool(name="perbatch", bufs=1))
        self.pb2_pool = ctx.enter_context(tc.tile_pool(name="perbatch2", bufs=2))
        self.mm_ps = ctx.enter_context(tc.tile_pool(name="mm_ps", bufs=4, space="PSUM"))
        self.zt_ps = ctx.enter_context(tc.tile_pool(name="zt_ps", bufs=2, space="PSUM"))
        self.lg_ps = ctx.enter_context(tc.tile_pool(name="lg_ps", bufs=2, space="PSUM"))

    # ---------------- per-batch text preamble: textT, te, kT, v ----------------
    def prebatch(self, b):
        nc = self.nc
        st = self.st[b]
        text_nat = self.pb_pool.tile([Q, D], F32R, tag="text_nat", name=f"text_nat{b}")
        nc.sync.dma_start(out=text_nat, in_=self.text.ap()[b])
        text_f = text_nat.bitcast(F32)

        textT = self.pb_pool.tile([128, DC, Q], F32R, tag="textT", name=f"textT{b}")
        for dc in range(DC):
            pt = self.mm_ps.tile([128, G], F32, tag="mm")
            nc.tensor.transpose(pt[:, :Q], text_f[:, ts(dc, 128)], self.ident)
            nc.scalar.copy(out=textT[:, dc, :], in_=pt[:, :Q])

        # text entropy -> evt (unnormalized te), S_t
        sm = self.sm_pool
        maxm = sm.tile([Q, 1], F32, tag="maxm")
        nc.vector.reduce_max(out=maxm, in_=text_f, axis=mybir.AxisListType.X)
        negm = sm.tile([Q, 1], F32, tag="negm")
        nc.vector.tensor_scalar_mul(out=negm, in0=maxm, scalar1=-1.0)
        et = self.es_pool.tile([Q, D], F32, tag="ex", name=f"et{b}")
        zt = sm.tile([Q, 1], F32, tag="zt")
        nc.scalar.activation(out=et, in_=text_f, func=AF.Exp, bias=negm, accum_out=zt)
        tt = sm.tile([Q, 1], F32, tag="tt")
        nc.vector.tensor_mul(out=et, in0=et, in1=text_f)
        nc.vector.reduce_sum(out=tt, in_=et, axis=mybir.AxisListType.X)
        rzt = sm.tile([Q, 1], F32, tag="rzt")
        nc.vector.reciprocal(out=rzt, in_=zt)
        t2 = sm.tile([Q, 1], F32, tag="t2")
        nc.vector.tensor_mul(out=t2, in0=tt, in1=rzt)
        lnz = sm.tile([Q, 1], F32, tag="lnz")
        nc.scalar.activation(out=lnz, in_=zt, func=AF.Ln)
        ent_t = sm.tile([Q, 1], F32, tag="ent_t")
        nc.vector.tensor_sub(out=ent_t, in0=lnz, in1=t2)
        nc.vector.tensor_add(out=ent_t, in0=ent_t, in1=maxm)
        evt = sm.tile([Q, 1], F32, tag="evt", name=f"evt{b}")
        nc.scalar.activation(out=evt, in_=ent_t, func=AF.Exp)
        st["evt"] = evt
        # S_t via DRAM round-trip (column -> row)
        nc.sync.dma_start(
            out=self.st_dram.ap()[b].rearrange("one p -> p one"), in_=evt)
        st_row = sm.tile([1, Q], F32, tag="st_row", name=f"strow{b}")
        nc.sync.dma_start(out=st_row, in_=self.st_dram.ap()[b])
        st_sb = sm.tile([1, 1], F32, tag="st_sb", name=f"stsb{b}")
        nc.vector.reduce_sum(out=st_sb, in_=st_row, axis=mybir.AxisListType.X)
        st["st_sb"] = st_sb

        # kT projection (fp8, j on partitions)
        kTb = self.pb2_pool.tile([128, DC, Q], FP8, tag="kTb", name=f"kTb{b}")
        for jc in range(DC):
            kp = self.mm_ps.tile([128, G], F32, tag="mm")
            for dc in range(DC):
                nc.tensor.matmul(
                    kp[:, :Q], self.wkT[:, dc, ts(jc, 128)], textT[:, dc, :],
                    start=(dc == 0), stop=(dc == DC - 1),
                )
            nc.scalar.activation(
                out=kTb[:, jc, :], in_=kp[:, :Q], func=AF.Identity,
                bias=self.bk_col[:, jc : jc + 1],
            )
        st["kTb"] = kTb

        # v projection (float32r, q on partitions)
        v_sb = self.pb2_pool.tile([Q, D], F32R, tag="v_sb", name=f"v{b}")
        for jg, jw in ((0, G), (1, D - G)):
            vp = self.mm_ps.tile([128, G], F32, tag="mm")
            for dc in range(DC):
                nc.tensor.matmul(
                    vp[:, :jw], textT[:, dc, :],
                    self.wvT[:, dc, jg * G : jg * G + jw],
                    start=(dc == 0), stop=False,
                )
            nc.tensor.matmul(
                vp[:, :jw], self.ones_row, self.bv_row[:, jg * G : jg * G + jw],
                start=False, stop=True,
            )
            nc.scalar.copy(out=v_sb[:, jg * G : jg * G + jw], in_=vp[:, :jw])
        st["v_sb"] = v_sb

        st["qT"] = self.pb2_pool.tile([128, DC, N], FP8, tag="qT", name=f"qT{b}")
        st["zc"] = self.pb_pool.tile([NG, G], F32, tag="zc", name=f"zc{b}")
        st["tcol"] = self.pb_pool.tile([NG, G], F32, tag="tcol", name=f"tcol{b}")

    # ---------------- phase 1 (per group): entropy partials + qT ----------------
    def phase1_group(self, b, g):
        nc = self.nc
        st = self.st[b]
        gs = slice(g * G, (g + 1) * G)
        vf = self.vf_pool.tile([128, DC, G], F32R, tag="vf")
        nc.sync.dma_start(
            out=vf,
            in_=self.visual.ap()[b].rearrange("(c p) n -> p c n", p=128)[:, :, gs],
        )
        vf_f = vf.bitcast(F32)

        zp = self.zt_ps.tile([1, G], F32, tag="zt")
        tp = self.zt_ps.tile([1, G], F32, tag="zt")
        for dc in range(DC):
            ex = self.es_pool.tile([128, G], F32R, tag="ex")
            nc.scalar.activation(out=ex, in_=vf_f[:, dc, :], func=AF.Exp)
            xe = self.es_pool.tile([128, G], F32R, tag="xe")
            nc.vector.tensor_mul(out=xe, in0=ex.bitcast(F32), in1=vf_f[:, dc, :])
            nc.tensor.matmul(zp, self.ones_col, ex,
                             start=(dc == 0), stop=(dc == DC - 1))
            nc.tensor.matmul(tp, self.ones_col, xe,
                             start=(dc == 0), stop=(dc == DC - 1))

        zrow = self.sm_pool.tile([1, G], F32, tag="zrow")
        nc.scalar.copy(out=zrow, in_=zp)
        nc.sync.dma_start(out=st["zc"][g : g + 1, :], in_=zrow)
        trow = self.sm_pool.tile([1, G], F32, tag="trow")
        nc.scalar.copy(out=trow, in_=tp)
        nc.sync.dma_start(out=st["tcol"][g : g + 1, :], in_=trow)

        for jc in range(DC):
            qp = self.mm_ps.tile([128, G], F32, tag="mm")
            for dc in range(DC):
                nc.tensor.matmul(
                    qp, self.wqT[:, dc, ts(jc, 128)], vf[:, dc, :],
                    start=(dc == 0), stop=(dc == DC - 1),
                )
            nc.vector.tensor_scalar_add(
                out=st["qT"][:, jc, gs], in0=qp,
                scalar1=self.bq_col[:, jc : jc + 1],
            )

    # ---------------- per-batch entropy finalize ----------------
    def finalize(self, b):
        nc = self.nc
        st = self.st[b]
        zc, tcol = st["zc"], st["tcol"]
        rz = self.pb_pool.tile([NG, G], F32, tag="rz", name=f"rz{b}")
        nc.vector.reciprocal(out=rz, in_=zc)
        nc.vector.tensor_mul(out=rz, in0=tcol, in1=rz)
        nc.scalar.activation(out=zc, in_=zc, func=AF.Ln)
        nc.vector.tensor_sub(out=zc, in0=zc, in1=rz)
        exp_ent = self.pb_pool.tile([NG, G], F32R, tag="exp_ent", name=f"ee{b}")
        nc.scalar.activation(out=exp_ent, in_=zc, func=AF.Exp)
        nc.sync.dma_start(out=self.ve_dram.ap()[b], in_=exp_ent.bitcast(F32))

        sve_p = self.zt_ps.tile([1, G], F32, tag="zt")
        nc.tensor.matmul(sve_p, self.ones_col[:NG], exp_ent, start=True, stop=True)
        sve_sb = self.sm_pool.tile([1, 1], F32, tag="sve_sb", name=f"sve{b}")
        nc.vector.reduce_sum(out=sve_sb, in_=sve_p, axis=mybir.AxisListType.X)

        c0 = self.sm_pool.tile([1, 1], F32, tag="c0", name=f"c0{b}")
        nc.vector.tensor_mul(out=c0, in0=st["st_sb"], in1=sve_sb)
        nc.vector.reciprocal(out=c0, in_=c0)
        nc.vector.tensor_scalar_mul(out=c0, in0=c0, scalar1=1.0 / SQRT_D)
        nc.sync.dma_start(out=self.c0_dram.ap()[b], in_=c0)
        c0b = self.sm_pool.tile([128, 1], F32, tag="c0b", name=f"c0b{b}")
        nc.sync.dma_start(out=c0b, in_=self.c0_dram.ap()[b].broadcast_to((128, 1)))
        te_eff = self.pb2_pool.tile([Q, 1], F32, tag="te_eff", name=f"te{b}")
        nc.vector.tensor_mul(out=te_eff, in0=st["evt"], in1=c0b)
        st["te_eff"] = te_eff

    # ---------------- phase 2 (per group): attention ----------------
    def phase2_group(self, b, g):
        nc = self.nc
        st = self.st[b]
        gs = slice(g * G, (g + 1) * G)
        veb = self.at_pool.tile([128, G], F32, tag="veb", bufs=4)
        nc.sync.dma_start(
            out=veb, in_=self.ve_dram.ap()[b][g : g + 1, :].broadcast_to((128, G))
        )

        lp = self.lg_ps.tile([Q, G], F32, tag="lg")
        for jc in range(DC):
            nc.tensor.matmul(
                lp, st["kTb"][:, jc, :], st["qT"][:, jc, gs],
                start=(jc == 0), stop=(jc == DC - 1),
            )
        smod = self.at_pool.tile([Q, G], F32, tag="smod")
        nc.vector.tensor_mul(out=smod, in0=lp, in1=veb)
        ea = self.at_pool.tile([Q, G], F32R, tag="ea")
        nc.scalar.activation(out=ea, in_=smod, func=AF.Exp, scale=st["te_eff"])

        zap = self.zt_ps.tile([1, G], F32, tag="zt")
        nc.tensor.matmul(zap, self.ones_col, ea, start=True, stop=True)
        zarow = self.sm_pool.tile([1, G], F32R, tag="zarow")
        nc.scalar.copy(out=zarow, in_=zap)
        zb = self.lg_ps.tile([128, G], F32, tag="lg")
        nc.tensor.matmul(zb, self.ones_row, zarow, start=True, stop=True)
        rzb = self.at_pool.tile([128, G], F32, tag="rzb")
        nc.vector.reciprocal(out=rzb, in_=zb)
        # fold 1/Za into the attention weights once (vs 6 per-j evac muls)
        ean = self.at_pool.tile([Q, G], F32R, tag="smod")
        nc.vector.tensor_mul(out=ean, in0=ea.bitcast(F32), in1=rzb)

        for jh in range(2):
            oc = self.oc_pool.tile([128, DC // 2, G], F32, tag="oc")
            for jx in range(DC // 2):
                jc = jh * (DC // 2) + jx
                ep = self.mm_ps.tile([128, G], F32, tag="mm")
                nc.tensor.matmul(ep, st["v_sb"][:, ts(jc, 128)], ean,
                                 start=True, stop=True)
                nc.scalar.copy(out=oc[:, jx, :], in_=ep)
            nc.sync.dma_start(
                out=self.out.ap()[b].rearrange("(c p) n -> p c n", p=128)[
                    :, jh * (DC // 2) : (jh + 1) * (DC // 2), gs
                ],
                in_=oc,
            )


_compiled = {}


def kernel(**inputs):
    visual_feat = np.ascontiguousarray(inputs["visual_feat"], dtype=np.float32)
    text_feat = np.ascontiguousarray(inputs["text_feat"], dtype=np.float32)
    Wq = np.ascontiguousarray(inputs["Wq"], dtype=np.float32)
    Wk = np.ascontiguousarray(inputs["Wk"], dtype=np.float32)
    Wv = np.ascontiguousarray(inputs["Wv"], dtype=np.float32)
    bq = np.ascontiguousarray(inputs["bq"], dtype=np.float32)
    bk = np.ascontiguousarray(inputs["bk"], dtype=np.float32)
    bv = np.ascontiguousarray(inputs["bv"], dtype=np.float32)

    vis = visual_feat.reshape(B, D, N)
    in_maps = []
    for c in range(N_CORES):
        bs = slice(c * BPC, (c + 1) * BPC)
        in_maps.append(
            {
                "visual": np.ascontiguousarray(vis[bs]),
                "text": np.ascontiguousarray(text_feat[bs]),
                "wq": Wq, "wk": Wk, "wv": Wv,
                "bq": bq, "bk": bk, "bv": bv,
            }
        )

    if "nc" not in _compiled:
        nc = build_bass()
        nc.compile()
        _compiled["nc"] = nc
    res = run_bass_kernel_spmd(_compiled["nc"], in_maps, core_ids=list(range(N_CORES)))
    _compiled["last_result"] = res

    out = np.concatenate([r["out"] for r in res.results], axis=0)
    return out.reshape(B, D, HH, WW)


if __name__ == "__main__":
    nc = build_bass()
    nc.compile()
    print("build ok")
